# revision 31
# baseline (speedup 1.0000x reference)
"""Trainium2 Bass kernel for nn_AttentionFusion (B=8192, M=4, H=1024), 8-core data parallel.

Math (exact reformulation of the reference):
  logits[b,m,n] = conf[b,m] * (y_{4b+m} . x_{4b+n}) + conf[b,m]*beta[4b+n]
      (alpha/d rank-1 terms dropped: softmax over n is shift-invariant)
      with Y = X G, G = (Wq/sqrt(H))^T Wk, beta = X (Wk^T bq/32)
  wt[b,n] = sum_m softmax_n(logits)[b,m,n]
  Z[b]    = sum_n wt[b,n] X[4b+n]
  out[b]  = Z[b] (Wo Wv / 4)^T  (+ bias, added on host)

v3: score path (Y = X G, gram S = Y X^T) in fp8 e4m3 with DoubleRow double-pumping;
value path in fp16 (STT convex combine on the DVE with weights straight out of softmax,
XBAR-DMA transpose of Z, fp16 output projection paired across super-tiles).
All DRAM tensors are host-prearranged so every streaming DMA is one contiguous
burst per partition, and DMA kicks are spread across the SP / Activation / Pool
queues (each kick costs ~0.6 us of queue time).
"""
import sys

if '/opt/trn_rl_repo' not in sys.path:
    sys.path.insert(0, '/opt/trn_rl_repo')

import numpy as np
import ml_dtypes

B, M, H = 8192, 4, 1024
NCORES = 8
B_CORE = B // NCORES            # 1024 batch rows per core
T_CORE = B_CORE * M             # 4096 tokens per core
T_SUPER = 512                   # tokens per super-tile (128 batch rows)
P = 128
OC = H // P                     # 8 output chunks
HC = H // P                     # 8 contraction chunks
F8 = ml_dtypes.float8_e4m3      # TRN e4m3: max normal 240
F16 = np.float16

_NC_CACHE = {}


def build_bass(n_super=T_CORE // T_SUPER):
    import concourse.bass as bass
    import concourse.mybir as mybir
    import concourse.tile as tile
    from concourse import bacc

    assert n_super % 2 == 0
    n_pair = n_super // 2
    t_core = n_super * T_SUPER
    b_core = t_core // M
    b_super = T_SUPER // M                 # 128 batch rows per super-tile

    nc = bacc.Bacc(None, target_bir_lowering=False)
    # all host-prearranged to partition-major contiguous layouts
    xT4 = nc.dram_tensor("xT4", [P, n_super, HC, T_SUPER], mybir.dt.float8e4,
                         kind="ExternalInput")
    xg = nc.dram_tensor("xg", [b_core, M * H], mybir.dt.float16, kind="ExternalInput")
    wg2 = nc.dram_tensor("wg2", [P, OC, HC, P], mybir.dt.float8e4,
                         kind="ExternalInput")
    wc2 = nc.dram_tensor("wc2", [P, OC, HC, P], mybir.dt.float16,
                         kind="ExternalInput")
    conf2 = nc.dram_tensor("conf2", [P, n_super, M], mybir.dt.float32,
                           kind="ExternalInput")
    cb2 = nc.dram_tensor("cb2", [P, n_super, 16], mybir.dt.float32,
                         kind="ExternalInput")
    syv = nc.dram_tensor("syv", [1], mybir.dt.float32, kind="ExternalInput")
    outT3 = nc.dram_tensor("outT3", [P, n_pair, OC, 2 * b_super], mybir.dt.float16,
                           kind="ExternalOutput")

    FDT = mybir.dt.float32
    DT16 = mybir.dt.float16
    DT8 = mybir.dt.float8e4
    BDT = mybir.dt.bfloat16
    AX = mybir.AxisListType.X
    MUL = mybir.AluOpType.mult
    ADD = mybir.AluOpType.add
    DR = mybir.MatmulPerfMode.DoubleRow
    COPY = mybir.ActivationFunctionType.Copy
    EXP = mybir.ActivationFunctionType.Exp

    from contextlib import ExitStack
    with tile.TileContext(nc) as tc:
        with ExitStack() as _es:
            wp = _es.enter_context(tc.tile_pool(name="wp", bufs=1))
            cp = _es.enter_context(tc.tile_pool(name="cp", bufs=1))
            xp = _es.enter_context(tc.tile_pool(name="xp", bufs=4))
            xgp = _es.enter_context(tc.tile_pool(name="xgp", bufs=5))
            yp = _es.enter_context(tc.tile_pool(name="yp", bufs=3))
            gpl = _es.enter_context(tc.tile_pool(name="gp", bufs=3))
            smp = _es.enter_context(tc.tile_pool(name="smp", bufs=3))
            zp = _es.enter_context(tc.tile_pool(name="zp", bufs=3))
            ztp = _es.enter_context(tc.tile_pool(name="ztp", bufs=3))
            osb = _es.enter_context(tc.tile_pool(name="osb", bufs=3))
            psp = _es.enter_context(tc.tile_pool(name="psp", bufs=3, space="PSUM"))
            psg = _es.enter_context(tc.tile_pool(name="psg", bufs=2, space="PSUM"))
            pso = _es.enter_context(tc.tile_pool(name="pso", bufs=3, space="PSUM"))
            drg = _es.enter_context(tc.tile_pool(name="drg", bufs=n_super, space="DRAM"))

            # ---- HAM warmup: dataless matmuls spin the PE to the warm clock
            #      while the lead-in DMAs stream ----
            wu = wp.tile([P, P], BDT, tag="warm", name="warm_sb")
            nc.vector.memset(wu[:], 1.0)
            wups = psg.tile([P, P], FDT, tag="gram_ps", name="warm_ps")
            for i in range(28):
                nc.tensor.matmul(wups[:], wu[:], wu[:],
                                 start=(i == 0), stop=(i == 27))
            wuo = wp.tile([P, P], FDT, tag="warmo", name="warm_out")
            nc.scalar.copy(wuo[:], wups[:])

            # ---- resident weights / constants ----
            # wg chunk 0 first: the first Y matmul only needs it + xt(0); the
            # remaining wg chunks are kicked after the first x loads
            wg_sb = wp.tile([P, OC, HC, P], DT8, tag="wg", name="wg_sb")
            wc_sb = wp.tile([P, OC, HC, P], DT16, tag="wc", name="wc_sb")
            nc.sync.dma_start(wg_sb[:, 0], wg2[:, 0])
            conf_sb = cp.tile([P, n_super, M], FDT, tag="conf", name="conf_sb")
            nc.scalar.dma_start(conf_sb[:], conf2[:])
            cb_sb = cp.tile([P, n_super, 16], FDT, tag="cb", name="cb_sb")
            nc.scalar.dma_start(cb_sb[:], cb2[:])
            sy_sb = cp.tile([P, 1], FDT, tag="sy", name="sy_sb")
            nc.scalar.dma_start(sy_sb[:], syv[:].partition_broadcast(P))

            def load_xt(s):
                # single kicks; the framework splits big DMAs across all queues
                xt = xp.tile([P, HC, T_SUPER], DT8, tag="xt")
                nc.sync.dma_start(xt[:], xT4[:, s])
                return xt

            def load_xg(s):
                xgt = xgp.tile([P, M * H], DT16, tag="xg")
                nc.scalar.dma_start(xgt[:], xg[s * b_super:(s + 1) * b_super])
                return xgt

            def y_proj(s, xt):
                """Y = X G in fp8 DoubleRow; quantize back to fp8 with scale sy."""
                yT = yp.tile([P, OC, T_SUPER], DT8, tag="yT")
                for oc in range(OC):
                    pt = psp.tile([P, T_SUPER], FDT, tag="proj")
                    for kk in range(HC // 2):
                        nc.tensor.matmul(
                            pt[:], wg_sb[:, oc, 2 * kk:2 * kk + 2, :],
                            xt[:, 2 * kk:2 * kk + 2, :],
                            start=(kk == 0), stop=(kk == HC // 2 - 1),
                            perf_mode=DR)
                    nc.scalar.activation(yT[:, oc, :], pt[:], COPY,
                                         scale=sy_sb[:])
                return yT

            def gram_part(s, xt, yT):
                """Block-diag scores -> DRAM bounce -> single-DMA diag gather."""
                gram_sb = gpl.tile([P, 4, P], FDT, tag="gram")
                for tt in range(4):
                    gps = psg.tile([P, P], FDT, tag="gram_ps")
                    tsl = slice(tt * P, (tt + 1) * P)
                    for kk in range(HC // 2):
                        nc.tensor.matmul(
                            gps[:], yT[:, 2 * kk:2 * kk + 2, tsl],
                            xt[:, 2 * kk:2 * kk + 2, tsl],
                            start=(kk == 0), stop=(kk == HC // 2 - 1),
                            perf_mode=DR)
                    nc.vector.tensor_copy(gram_sb[:, tt, :], gps[:])
                # bounce to DRAM with a 129-row tile pitch: 129*128 = 32*516, so
                # the (tile j, group g) diagonal walk becomes ONE uniform stride
                # of 516 and the whole 4x4-block gather is a single 3-dim DMA
                gram_dr = drg.tile([4, P + 1, P], FDT, tag="gram_dr",
                                   name=f"gram_dr{s}")
                nc.gpsimd.dma_start(gram_dr[:, 0:P, :].transpose([1, 0, 2]),
                                    gram_sb[:])
                s_sb = smp.tile([P, 16], FDT, tag="s", name="s_sb")
                base = gram_dr[:]
                src = bass.AP(base.tensor, base.offset,
                              [[4 * P + 4, P], [P, 4], [1, 4]])
                nc.gpsimd.dma_start(
                    s_sb[:].rearrange("p (m n) -> p m n", n=4), src)
                return s_sb

            def softmax_combine(s, s_sb, xgt, zb2):
                """softmax of the (iteration-old) gathered scores -> STT combine."""
                # logits = s_raw*conf_dev + conf*beta
                scl = smp.tile([P, 16], FDT, tag="scl", name="scl")
                for m in range(M):
                    nc.vector.scalar_tensor_tensor(
                        scl[:, 4 * m:4 * m + 4], s_sb[:, 4 * m:4 * m + 4],
                        conf_sb[:, s, m:m + 1], cb_sb[:, s, 4 * m:4 * m + 4],
                        op0=MUL, op1=ADD)
                ex = smp.tile([P, 16], FDT, tag="ex", name="ex")
                nc.scalar.activation(ex[:], scl[:], EXP)
                z4 = smp.tile([P, M], FDT, tag="z4", name="z4")
                nc.vector.reduce_sum(z4[:], ex[:].rearrange("p (m n) -> p m n", n=4),
                                     axis=AX)
                r4 = smp.tile([P, M], FDT, tag="r4", name="r4")
                nc.vector.reciprocal(r4[:], z4[:])
                w4 = smp.tile([P, M], FDT, tag="w4", name="w4")
                nc.vector.tensor_scalar_mul(w4[:], ex[:, 0:4], r4[:, 0:1])
                for m in range(1, M):
                    nc.vector.scalar_tensor_tensor(
                        w4[:], ex[:, 4 * m:4 * m + 4], r4[:, m:m + 1], w4[:],
                        op0=MUL, op1=ADD)
                # convex combine, batch-major, fp16 accumulate (eps ~1e-3)
                with nc.allow_low_precision(reason="convex combine, fp16 acc"):
                    zb = zb2[:, s % 2, :]
                    nc.vector.tensor_scalar_mul(zb, xgt[:, 0:H], w4[:, 0:1])
                    for n in range(1, M):
                        nc.vector.scalar_tensor_tensor(
                            zb, xgt[:, n * H:(n + 1) * H], w4[:, n:n + 1],
                            zb, op0=MUL, op1=ADD)

            def transpose_pair(zb2):
                zT2 = ztp.tile([P, 16, P], DT16, tag="zT2")
                nc.sync.dma_start_transpose(zT2[:], zb2[:].rearrange("p a b -> p (a b)"))
                # logical row r = pair*1024 + h  ->  partition h%128, chunk pair*8 + h//128
                return zT2

            def transpose_half(zb2, half, zT2=None):
                if zT2 is None:
                    zT2 = ztp.tile([P, 16, P], DT16, tag="zT2")
                nc.sync.dma_start_transpose(zT2[:, 8 * half:8 * (half + 1), :],
                                            zb2[:, half, :])
                return zT2

            def out_pair(pr, zT2):
                zv = zT2[:].rearrange("p (a c) b -> p c a b", a=2)
                o_sb = osb.tile([P, OC, 2 * b_super], DT16, tag="osb")
                for oc in range(OC):
                    po = pso.tile([P, 2 * b_super], FDT, tag="outp")
                    for hc in range(HC):
                        nc.tensor.matmul(
                            po[:], wc_sb[:, oc, hc, :], zv[:, hc],
                            start=(hc == 0), stop=(hc == HC - 1))
                    if oc % 2 == 0:
                        nc.scalar.copy(o_sb[:, oc, :], po[:])
                    else:
                        nc.vector.tensor_copy(o_sb[:, oc, :], po[:])
                    if oc == OC // 2 - 1:
                        # stream the first half out while the rest computes
                        nc.sync.dma_start(outT3[:, pr, 0:OC // 2], o_sb[:, 0:OC // 2])
                nc.sync.dma_start(outT3[:, pr, OC // 2:], o_sb[:, OC // 2:])

            # software pipeline: loads prefetched one super-tile ahead; the
            # transpose / output-write kicks are deferred so their data is a
            # full super-tile old by the time the SP queue reaches them (no
            # head-of-line blocking of the next x loads)
            xts = {0: load_xt(0)}
            xgs = {0: load_xg(0)}
            for oc in range(1, OC):
                nc.sync.dma_start(wg_sb[:, oc], wg2[:, oc])
            # wc needed first by out_pair(0) at iteration 5: spread the 2MB
            # load across iterations 1..4 to not crowd the x streams
            wc_iters = min(4, n_super - 1)
            wc_per = -(-OC // wc_iters)
            state = {}
            sss = {}
            zbs = {}
            zts = {}

            def do_sm(k):
                if k % 2 == 0:
                    zbs[k // 2] = zp.tile([P, 2, H], DT16, tag="zb2",
                                          name=f"zb2_{k // 2}")
                softmax_combine(k, sss.pop(k), xgs.pop(k), zbs[k // 2])

            for s in range(n_super):
                if s + 1 < n_super:
                    xts[s + 1] = load_xt(s + 1)
                yT = y_proj(s, xts[s])
                state[s] = (xts.pop(s), yT)
                if 1 <= s <= wc_iters:
                    for oc in range(wc_per * (s - 1), min(OC, wc_per * s)):
                        nc.scalar.dma_start(wc_sb[:, oc], wc2[:, oc])
                if s >= 1:
                    sss[s - 1] = gram_part(s - 1, *state.pop(s - 1))
                if s >= 2:
                    do_sm(s - 2)
                if s >= 4 and s % 2 == 0:
                    zts[(s - 4) // 2] = transpose_pair(zbs.pop((s - 4) // 2))
                if s >= 5 and s % 2 == 1:
                    out_pair((s - 5) // 2, zts.pop((s - 5) // 2))
                if s == n_super - 1 and n_super >= 2:
                    do_sm(s - 1)          # collapse pipeline at the end
                if s + 1 < n_super:
                    xgs[s + 1] = load_xg(s + 1)
            sl = n_super - 1
            sss[sl] = gram_part(sl, *state.pop(sl))
            if n_pair >= 2:
                zts[n_pair - 2] = transpose_pair(zbs.pop(n_pair - 2))
            # last pair: the even half's combine is already done; transpose it
            # while the odd half's softmax runs, then only the odd half's
            # transpose sits on the tail critical path
            ztl = transpose_half(zbs[sl // 2], 0)
            do_sm(sl)
            transpose_half(zbs.pop(sl // 2), 1, ztl)
            if n_pair >= 2:
                out_pair(n_pair - 2, zts.pop(n_pair - 2))
            out_pair(n_pair - 1, ztl)
    nc.compile()
    return nc


def _get_nc(n_super=T_CORE // T_SUPER):
    if n_super not in _NC_CACHE:
        _NC_CACHE[n_super] = build_bass(n_super)
    return _NC_CACHE[n_super]


def prep_in_maps(inputs, ncores=NCORES):
    """Host-side: fold weights, pick fp8 scales, prearrange layouts, cast."""
    f32 = np.float32
    f64 = np.float64
    feats = np.asarray(inputs["features"], f32)
    confs = np.asarray(inputs["confidences"], f32).reshape(-1, M)
    Wq = np.asarray(inputs["Wq"], f64)
    Wk = np.asarray(inputs["Wk"], f64)
    Wv = np.asarray(inputs["Wv"], f64)
    Wo = np.asarray(inputs["Wo"], f64)
    bq = np.asarray(inputs["bq"], f64)
    bv = np.asarray(inputs["bv"], f64)
    bo = np.asarray(inputs["bo"], f64)

    s = 1.0 / np.sqrt(H)
    G = (Wq * s).T @ Wk                         # [h, h']
    WcT = ((Wo @ Wv) / 4.0).T                   # [f, o]
    bc_h = (bv @ Wo.T + bo).astype(f32)         # added on host after the run
    w_vec = Wk.T @ (bq * s)                     # beta = X w_vec

    nb = feats.shape[0]
    b_core = nb // ncores
    t_core = b_core * M
    n_super = t_core // T_SUPER
    X = feats.reshape(nb * M, H)

    sx = f32(224.0 / np.abs(X).max())
    sg = f32(224.0 / np.abs(G).max())
    X8 = (X * sx).astype(F8)                    # [t, h]
    wg_f8 = (G * sg).astype(F8)                 # [k, o]
    col_sig = np.sqrt((np.asarray(wg_f8, f32) ** 2).sum(axis=0)).max() * sx
    sy = f32(224.0 / (6.5 * col_sig))           # fp8-Y sigma ~34, 6.5-sigma headroom
    descale = f32(1.0) / (f32(sx) * f32(sx) * f32(sg) * f32(sy))

    # prearranged weights: [p, oc, c, o'] with k = c*128+p, o = oc*128+o'
    wg2_h = np.ascontiguousarray(
        wg_f8.reshape(HC, P, OC, P).transpose(1, 2, 0, 3))
    wc2_h = np.ascontiguousarray(
        WcT.astype(F16).reshape(HC, P, OC, P).transpose(1, 2, 0, 3))

    beta = (X @ w_vec.astype(f32)).astype(f32)  # [nb*M]
    conf_dev = confs * descale
    cbeta = confs[:, :, None] * beta.reshape(nb, M)[:, None, :]   # [b, m, n]
    cb16_h = cbeta.reshape(nb, 16).astype(f32)
    xg_h = feats.reshape(nb, M * H).astype(F16)

    in_maps = []
    for c in range(ncores):
        tsl = slice(c * t_core, (c + 1) * t_core)
        bsl = slice(c * b_core, (c + 1) * b_core)
        # xT4 [p, s, c, t]: X8[(s t), (c p)] -> transpose
        xT4_h = np.ascontiguousarray(
            X8[tsl].reshape(n_super, T_SUPER, HC, P).transpose(3, 0, 2, 1))
        conf2_h = np.ascontiguousarray(
            conf_dev[bsl].reshape(n_super, P, M).transpose(1, 0, 2))
        cb2_h = np.ascontiguousarray(
            cb16_h[bsl].reshape(n_super, P, 16).transpose(1, 0, 2))
        in_maps.append({
            "xT4": xT4_h,
            "xg": np.ascontiguousarray(xg_h[bsl]),
            "wg2": wg2_h, "wc2": wc2_h,
            "conf2": conf2_h, "cb2": cb2_h,
            "syv": np.array([sy], f32),
        })
    return in_maps, bc_h


def install_ntff_hook():
    """Best-effort shim so run_bass_kernel_spmd(trace=True) can profile under axon."""
    import types
    try:
        from antenv.axon_hooks import get_axon_ntff_profile_hook  # noqa: F401
        return True
    except ImportError:
        pass
    try:
        import antenv
        mod = types.ModuleType("antenv.axon_hooks")
        _state = {"hook": None}
        mod.set_axon_ntff_profile_hook = lambda h: _state.__setitem__("hook", h)
        mod.get_axon_ntff_profile_hook = lambda: _state["hook"]
        sys.modules["antenv.axon_hooks"] = mod
        antenv.axon_hooks = mod
        from trn_agent_boot.trn_boot import _ntff_profile_via_ctypes
        hook = _ntff_profile_via_ctypes('/opt/axon/libaxon_pjrt.so')
        if hook is None:
            return False
        mod.set_axon_ntff_profile_hook(hook)
        return True
    except Exception:
        return False


def run(inputs, trace=False, tmpdir=None):
    """Run the 8-core kernel; returns (out [B, H] f32, BassKernelResults)."""
    from concourse.bass_utils import run_bass_kernel_spmd
    nc = _get_nc()
    in_maps, bc_h = prep_in_maps(inputs)
    if trace:
        install_ntff_hook()
    res = run_bass_kernel_spmd(nc, in_maps, core_ids=list(range(NCORES)),
                               trace=trace, tmpdir=tmpdir)
    # outT3 [p, pr, oc, b2]: out[b, o] with b = (2pr + b2//128)*128 + b2%128,
    # o = oc*128 + p
    outs = []
    for o in res.results:
        o3 = np.asarray(o["outT3"], np.float32)     # [128, n_pair, 8, 256]
        npair = o3.shape[1]
        o4 = o3.reshape(P, npair, OC, 2, 128)       # [p, pr, oc, half, 128]
        # -> [pr, half, b(128), oc, p]
        out_c = o4.transpose(1, 3, 4, 2, 0).reshape(npair * 2 * 128, H)
        outs.append(out_c)
    out = np.concatenate(outs, axis=0)
    out += bc_h[None, :]
    return out, res


def kernel(**inputs):
    out, _ = run(inputs, trace=False)
    return out


# revision 32
# speedup vs baseline: 1.0049x; 1.0049x over previous
"""Trainium2 Bass kernel for nn_AttentionFusion (B=8192, M=4, H=1024), 8-core data parallel.

Math (exact reformulation of the reference):
  logits[b,m,n] = conf[b,m] * (y_{4b+m} . x_{4b+n}) + conf[b,m]*beta[4b+n]
      (alpha/d rank-1 terms dropped: softmax over n is shift-invariant)
      with Y = X G, G = (Wq/sqrt(H))^T Wk, beta = X (Wk^T bq/32)
  wt[b,n] = sum_m softmax_n(logits)[b,m,n]
  Z[b]    = sum_n wt[b,n] X[4b+n]
  out[b]  = Z[b] (Wo Wv / 4)^T  (+ bias, added on host)

v3: score path (Y = X G, gram S = Y X^T) in fp8 e4m3 with DoubleRow double-pumping;
value path in fp16 (STT convex combine on the DVE with weights straight out of softmax,
XBAR-DMA transpose of Z, fp16 output projection paired across super-tiles).
All DRAM tensors are host-prearranged so every streaming DMA is one contiguous
burst per partition, and DMA kicks are spread across the SP / Activation / Pool
queues (each kick costs ~0.6 us of queue time).
"""
import sys

if '/opt/trn_rl_repo' not in sys.path:
    sys.path.insert(0, '/opt/trn_rl_repo')

import numpy as np
import ml_dtypes

B, M, H = 8192, 4, 1024
NCORES = 8
B_CORE = B // NCORES            # 1024 batch rows per core
T_CORE = B_CORE * M             # 4096 tokens per core
T_SUPER = 512                   # tokens per super-tile (128 batch rows)
P = 128
OC = H // P                     # 8 output chunks
HC = H // P                     # 8 contraction chunks
F8 = ml_dtypes.float8_e4m3      # TRN e4m3: max normal 240
F16 = np.float16

_NC_CACHE = {}


def build_bass(n_super=T_CORE // T_SUPER):
    import concourse.bass as bass
    import concourse.mybir as mybir
    import concourse.tile as tile
    from concourse import bacc

    assert n_super % 2 == 0
    n_pair = n_super // 2
    t_core = n_super * T_SUPER
    b_core = t_core // M
    b_super = T_SUPER // M                 # 128 batch rows per super-tile

    nc = bacc.Bacc(None, target_bir_lowering=False)
    # all host-prearranged to partition-major contiguous layouts
    xT4 = nc.dram_tensor("xT4", [P, n_super, HC, T_SUPER], mybir.dt.float8e4,
                         kind="ExternalInput")
    xg = nc.dram_tensor("xg", [b_core, M * H], mybir.dt.float16, kind="ExternalInput")
    wg2 = nc.dram_tensor("wg2", [P, OC, HC, P], mybir.dt.float8e4,
                         kind="ExternalInput")
    wc2 = nc.dram_tensor("wc2", [P, OC, HC, P], mybir.dt.float16,
                         kind="ExternalInput")
    conf2 = nc.dram_tensor("conf2", [P, n_super, M], mybir.dt.float32,
                           kind="ExternalInput")
    cb2 = nc.dram_tensor("cb2", [P, n_super, 16], mybir.dt.float32,
                         kind="ExternalInput")
    syv = nc.dram_tensor("syv", [1], mybir.dt.float32, kind="ExternalInput")
    outT3 = nc.dram_tensor("outT3", [P, n_pair, OC, 2 * b_super], mybir.dt.float16,
                           kind="ExternalOutput")

    FDT = mybir.dt.float32
    DT16 = mybir.dt.float16
    DT8 = mybir.dt.float8e4
    BDT = mybir.dt.bfloat16
    AX = mybir.AxisListType.X
    MUL = mybir.AluOpType.mult
    ADD = mybir.AluOpType.add
    DR = mybir.MatmulPerfMode.DoubleRow
    COPY = mybir.ActivationFunctionType.Copy
    EXP = mybir.ActivationFunctionType.Exp

    from contextlib import ExitStack
    with tile.TileContext(nc) as tc:
        with ExitStack() as _es:
            wp = _es.enter_context(tc.tile_pool(name="wp", bufs=1))
            cp = _es.enter_context(tc.tile_pool(name="cp", bufs=1))
            xp = _es.enter_context(tc.tile_pool(name="xp", bufs=4))
            xgp = _es.enter_context(tc.tile_pool(name="xgp", bufs=4))
            yp = _es.enter_context(tc.tile_pool(name="yp", bufs=3))
            gpl = _es.enter_context(tc.tile_pool(name="gp", bufs=3))
            smp = _es.enter_context(tc.tile_pool(name="smp", bufs=3))
            zp = _es.enter_context(tc.tile_pool(name="zp", bufs=3))
            ztp = _es.enter_context(tc.tile_pool(name="ztp", bufs=3))
            osb = _es.enter_context(tc.tile_pool(name="osb", bufs=3))
            psp = _es.enter_context(tc.tile_pool(name="psp", bufs=3, space="PSUM"))
            psg = _es.enter_context(tc.tile_pool(name="psg", bufs=2, space="PSUM"))
            pso = _es.enter_context(tc.tile_pool(name="pso", bufs=3, space="PSUM"))
            drg = _es.enter_context(tc.tile_pool(name="drg", bufs=n_super, space="DRAM"))

            # ---- HAM warmup: dataless matmuls spin the PE to the warm clock
            #      while the lead-in DMAs stream ----
            wu = wp.tile([P, P], BDT, tag="warm", name="warm_sb")
            nc.vector.memset(wu[:], 1.0)
            wups = psg.tile([P, P], FDT, tag="gram_ps", name="warm_ps")
            for i in range(28):
                nc.tensor.matmul(wups[:], wu[:], wu[:],
                                 start=(i == 0), stop=(i == 27))
            wuo = wp.tile([P, P], FDT, tag="warmo", name="warm_out")
            nc.scalar.copy(wuo[:], wups[:])

            # ---- resident weights / constants ----
            # wg chunk 0 first: the first Y matmul only needs it + xt(0); the
            # remaining wg chunks are kicked after the first x loads
            wg_sb = wp.tile([P, OC, HC, P], DT8, tag="wg", name="wg_sb")
            wc_sb = wp.tile([P, OC, HC, P], DT16, tag="wc", name="wc_sb")
            nc.sync.dma_start(wg_sb[:, 0], wg2[:, 0])
            conf_sb = cp.tile([P, n_super, M], FDT, tag="conf", name="conf_sb")
            nc.scalar.dma_start(conf_sb[:], conf2[:])
            cb_sb = cp.tile([P, n_super, 16], FDT, tag="cb", name="cb_sb")
            nc.scalar.dma_start(cb_sb[:], cb2[:])
            sy_sb = cp.tile([P, 1], FDT, tag="sy", name="sy_sb")
            nc.scalar.dma_start(sy_sb[:], syv[:].partition_broadcast(P))

            def load_xt(s):
                # single kicks; the framework splits big DMAs across all queues
                xt = xp.tile([P, HC, T_SUPER], DT8, tag="xt")
                nc.sync.dma_start(xt[:], xT4[:, s])
                return xt

            def load_xg(s):
                xgt = xgp.tile([P, M * H], DT16, tag="xg")
                nc.scalar.dma_start(xgt[:], xg[s * b_super:(s + 1) * b_super])
                return xgt

            def y_proj(s, xt):
                """Y = X G in fp8 DoubleRow; quantize back to fp8 with scale sy."""
                yT = yp.tile([P, OC, T_SUPER], DT8, tag="yT")
                for oc in range(OC):
                    pt = psp.tile([P, T_SUPER], FDT, tag="proj")
                    for kk in range(HC // 2):
                        nc.tensor.matmul(
                            pt[:], wg_sb[:, oc, 2 * kk:2 * kk + 2, :],
                            xt[:, 2 * kk:2 * kk + 2, :],
                            start=(kk == 0), stop=(kk == HC // 2 - 1),
                            perf_mode=DR)
                    nc.scalar.activation(yT[:, oc, :], pt[:], COPY,
                                         scale=sy_sb[:])
                return yT

            def gram_softmax(s, xt, yT, xgt, zb2):
                """Block-diag scores -> softmax -> STT combine into zb2[:, s%2]."""
                gram_sb = gpl.tile([P, 4, P], FDT, tag="gram")
                for tt in range(4):
                    gps = psg.tile([P, P], FDT, tag="gram_ps")
                    tsl = slice(tt * P, (tt + 1) * P)
                    for kk in range(HC // 2):
                        nc.tensor.matmul(
                            gps[:], yT[:, 2 * kk:2 * kk + 2, tsl],
                            xt[:, 2 * kk:2 * kk + 2, tsl],
                            start=(kk == 0), stop=(kk == HC // 2 - 1),
                            perf_mode=DR)
                    nc.vector.tensor_copy(gram_sb[:, tt, :], gps[:])
                # bounce to DRAM with a 129-row tile pitch: 129*128 = 32*516, so
                # the (tile j, group g) diagonal walk becomes ONE uniform stride
                # of 516 and the whole 4x4-block gather is a single 3-dim DMA
                gram_dr = drg.tile([4, P + 1, P], FDT, tag="gram_dr",
                                   name=f"gram_dr{s}")
                nc.gpsimd.dma_start(gram_dr[:, 0:P, :].transpose([1, 0, 2]),
                                    gram_sb[:])
                s_sb = smp.tile([P, 16], FDT, tag="s", name="s_sb")
                base = gram_dr[:]
                src = bass.AP(base.tensor, base.offset,
                              [[4 * P + 4, P], [P, 4], [1, 4]])
                nc.gpsimd.dma_start(
                    s_sb[:].rearrange("p (m n) -> p m n", n=4), src)
                # logits = s_raw*conf_dev + conf*beta
                scl = smp.tile([P, 16], FDT, tag="scl", name="scl")
                for m in range(M):
                    nc.vector.scalar_tensor_tensor(
                        scl[:, 4 * m:4 * m + 4], s_sb[:, 4 * m:4 * m + 4],
                        conf_sb[:, s, m:m + 1], cb_sb[:, s, 4 * m:4 * m + 4],
                        op0=MUL, op1=ADD)
                ex = smp.tile([P, 16], FDT, tag="ex", name="ex")
                nc.scalar.activation(ex[:], scl[:], EXP)
                z4 = smp.tile([P, M], FDT, tag="z4", name="z4")
                nc.vector.reduce_sum(z4[:], ex[:].rearrange("p (m n) -> p m n", n=4),
                                     axis=AX)
                r4 = smp.tile([P, M], FDT, tag="r4", name="r4")
                nc.vector.reciprocal(r4[:], z4[:])
                w4 = smp.tile([P, M], FDT, tag="w4", name="w4")
                nc.vector.tensor_scalar_mul(w4[:], ex[:, 0:4], r4[:, 0:1])
                for m in range(1, M):
                    nc.vector.scalar_tensor_tensor(
                        w4[:], ex[:, 4 * m:4 * m + 4], r4[:, m:m + 1], w4[:],
                        op0=MUL, op1=ADD)
                # convex combine, batch-major, fp16 accumulate (eps ~1e-3)
                with nc.allow_low_precision(reason="convex combine, fp16 acc"):
                    zb = zb2[:, s % 2, :]
                    nc.vector.tensor_scalar_mul(zb, xgt[:, 0:H], w4[:, 0:1])
                    for n in range(1, M):
                        nc.vector.scalar_tensor_tensor(
                            zb, xgt[:, n * H:(n + 1) * H], w4[:, n:n + 1],
                            zb, op0=MUL, op1=ADD)

            def transpose_pair(zb2):
                zT2 = ztp.tile([P, 16, P], DT16, tag="zT2")
                nc.sync.dma_start_transpose(zT2[:], zb2[:].rearrange("p a b -> p (a b)"))
                # logical row r = pair*1024 + h  ->  partition h%128, chunk pair*8 + h//128
                return zT2

            def transpose_half(zb2, half, zT2=None):
                if zT2 is None:
                    zT2 = ztp.tile([P, 16, P], DT16, tag="zT2")
                nc.sync.dma_start_transpose(zT2[:, 8 * half:8 * (half + 1), :],
                                            zb2[:, half, :])
                return zT2

            def out_pair(pr, zT2):
                zv = zT2[:].rearrange("p (a c) b -> p c a b", a=2)
                o_sb = osb.tile([P, OC, 2 * b_super], DT16, tag="osb")
                for oc in range(OC):
                    po = pso.tile([P, 2 * b_super], FDT, tag="outp")
                    for hc in range(HC):
                        nc.tensor.matmul(
                            po[:], wc_sb[:, oc, hc, :], zv[:, hc],
                            start=(hc == 0), stop=(hc == HC - 1))
                    if oc % 2 == 0:
                        nc.scalar.copy(o_sb[:, oc, :], po[:])
                    else:
                        nc.vector.tensor_copy(o_sb[:, oc, :], po[:])
                    if oc == OC // 2 - 1:
                        # stream the first half out while the rest computes
                        nc.sync.dma_start(outT3[:, pr, 0:OC // 2], o_sb[:, 0:OC // 2])
                nc.sync.dma_start(outT3[:, pr, OC // 2:], o_sb[:, OC // 2:])

            # software pipeline: loads prefetched one super-tile ahead; the
            # transpose / output-write kicks are deferred so their data is a
            # full super-tile old by the time the SP queue reaches them (no
            # head-of-line blocking of the next x loads)
            xts = {0: load_xt(0)}
            xgs = {0: load_xg(0)}
            for oc in range(1, OC):
                nc.sync.dma_start(wg_sb[:, oc], wg2[:, oc])
            # wc needed first by out_pair(0) at iteration 4: spread the 2MB
            # load across iterations 1..4 to not crowd the x streams
            wc_iters = min(4, n_super - 1)
            wc_per = -(-OC // wc_iters)
            state = {}
            zbs = {}
            zts = {}
            for s in range(n_super):
                if s + 1 < n_super:
                    xts[s + 1] = load_xt(s + 1)
                yT = y_proj(s, xts[s])
                state[s] = (xts.pop(s), yT, xgs.pop(s))
                if 1 <= s <= wc_iters:
                    for oc in range(wc_per * (s - 1), min(OC, wc_per * s)):
                        nc.scalar.dma_start(wc_sb[:, oc], wc2[:, oc])
                if s >= 1:
                    sp = s - 1
                    if sp % 2 == 0:
                        zbs[sp // 2] = zp.tile([P, 2, H], DT16, tag="zb2",
                                               name=f"zb2_{sp // 2}")
                    gram_softmax(sp, *state.pop(sp), zbs[sp // 2])
                if s >= 3 and s % 2 == 1:
                    pr = (s - 3) // 2
                    zts[pr] = transpose_pair(zbs.pop(pr))
                if s >= 4 and s % 2 == 0:
                    pr = (s - 4) // 2
                    out_pair(pr, zts.pop(pr))
                if s + 1 < n_super:
                    xgs[s + 1] = load_xg(s + 1)
            sl = n_super - 1
            # last pair: transpose the even half as soon as its combine is done
            # so only the odd half's transpose sits on the tail critical path
            ztl = transpose_half(zbs[sl // 2], 0)
            gram_softmax(sl, *state.pop(sl), zbs[sl // 2])
            transpose_half(zbs.pop(sl // 2), 1, ztl)
            if n_pair >= 2:
                out_pair(n_pair - 2, zts.pop(n_pair - 2))
            out_pair(n_pair - 1, ztl)
    nc.compile()
    return nc


def _get_nc(n_super=T_CORE // T_SUPER):
    if n_super not in _NC_CACHE:
        _NC_CACHE[n_super] = build_bass(n_super)
    return _NC_CACHE[n_super]


def prep_in_maps(inputs, ncores=NCORES):
    """Host-side: fold weights, pick fp8 scales, prearrange layouts, cast."""
    f32 = np.float32
    f64 = np.float64
    feats = np.asarray(inputs["features"], f32)
    confs = np.asarray(inputs["confidences"], f32).reshape(-1, M)
    Wq = np.asarray(inputs["Wq"], f64)
    Wk = np.asarray(inputs["Wk"], f64)
    Wv = np.asarray(inputs["Wv"], f64)
    Wo = np.asarray(inputs["Wo"], f64)
    bq = np.asarray(inputs["bq"], f64)
    bv = np.asarray(inputs["bv"], f64)
    bo = np.asarray(inputs["bo"], f64)

    s = 1.0 / np.sqrt(H)
    G = (Wq * s).T @ Wk                         # [h, h']
    WcT = ((Wo @ Wv) / 4.0).T                   # [f, o]
    bc_h = (bv @ Wo.T + bo).astype(f32)         # added on host after the run
    w_vec = Wk.T @ (bq * s)                     # beta = X w_vec

    nb = feats.shape[0]
    b_core = nb // ncores
    t_core = b_core * M
    n_super = t_core // T_SUPER
    X = feats.reshape(nb * M, H)

    sx = f32(224.0 / np.abs(X).max())
    sg = f32(224.0 / np.abs(G).max())
    X8 = (X * sx).astype(F8)                    # [t, h]
    wg_f8 = (G * sg).astype(F8)                 # [k, o]
    col_sig = np.sqrt((np.asarray(wg_f8, f32) ** 2).sum(axis=0)).max() * sx
    sy = f32(224.0 / (6.5 * col_sig))           # fp8-Y sigma ~34, 6.5-sigma headroom
    descale = f32(1.0) / (f32(sx) * f32(sx) * f32(sg) * f32(sy))

    # prearranged weights: [p, oc, c, o'] with k = c*128+p, o = oc*128+o'
    wg2_h = np.ascontiguousarray(
        wg_f8.reshape(HC, P, OC, P).transpose(1, 2, 0, 3))
    wc2_h = np.ascontiguousarray(
        WcT.astype(F16).reshape(HC, P, OC, P).transpose(1, 2, 0, 3))

    beta = (X @ w_vec.astype(f32)).astype(f32)  # [nb*M]
    conf_dev = confs * descale
    cbeta = confs[:, :, None] * beta.reshape(nb, M)[:, None, :]   # [b, m, n]
    cb16_h = cbeta.reshape(nb, 16).astype(f32)
    xg_h = feats.reshape(nb, M * H).astype(F16)

    in_maps = []
    for c in range(ncores):
        tsl = slice(c * t_core, (c + 1) * t_core)
        bsl = slice(c * b_core, (c + 1) * b_core)
        # xT4 [p, s, c, t]: X8[(s t), (c p)] -> transpose
        xT4_h = np.ascontiguousarray(
            X8[tsl].reshape(n_super, T_SUPER, HC, P).transpose(3, 0, 2, 1))
        conf2_h = np.ascontiguousarray(
            conf_dev[bsl].reshape(n_super, P, M).transpose(1, 0, 2))
        cb2_h = np.ascontiguousarray(
            cb16_h[bsl].reshape(n_super, P, 16).transpose(1, 0, 2))
        in_maps.append({
            "xT4": xT4_h,
            "xg": np.ascontiguousarray(xg_h[bsl]),
            "wg2": wg2_h, "wc2": wc2_h,
            "conf2": conf2_h, "cb2": cb2_h,
            "syv": np.array([sy], f32),
        })
    return in_maps, bc_h


def install_ntff_hook():
    """Best-effort shim so run_bass_kernel_spmd(trace=True) can profile under axon."""
    import types
    try:
        from antenv.axon_hooks import get_axon_ntff_profile_hook  # noqa: F401
        return True
    except ImportError:
        pass
    try:
        import antenv
        mod = types.ModuleType("antenv.axon_hooks")
        _state = {"hook": None}
        mod.set_axon_ntff_profile_hook = lambda h: _state.__setitem__("hook", h)
        mod.get_axon_ntff_profile_hook = lambda: _state["hook"]
        sys.modules["antenv.axon_hooks"] = mod
        antenv.axon_hooks = mod
        from trn_agent_boot.trn_boot import _ntff_profile_via_ctypes
        hook = _ntff_profile_via_ctypes('/opt/axon/libaxon_pjrt.so')
        if hook is None:
            return False
        mod.set_axon_ntff_profile_hook(hook)
        return True
    except Exception:
        return False


def run(inputs, trace=False, tmpdir=None):
    """Run the 8-core kernel; returns (out [B, H] f32, BassKernelResults)."""
    from concourse.bass_utils import run_bass_kernel_spmd
    nc = _get_nc()
    in_maps, bc_h = prep_in_maps(inputs)
    if trace:
        install_ntff_hook()
    res = run_bass_kernel_spmd(nc, in_maps, core_ids=list(range(NCORES)),
                               trace=trace, tmpdir=tmpdir)
    # outT3 [p, pr, oc, b2]: out[b, o] with b = (2pr + b2//128)*128 + b2%128,
    # o = oc*128 + p
    outs = []
    for o in res.results:
        o3 = np.asarray(o["outT3"], np.float32)     # [128, n_pair, 8, 256]
        npair = o3.shape[1]
        o4 = o3.reshape(P, npair, OC, 2, 128)       # [p, pr, oc, half, 128]
        # -> [pr, half, b(128), oc, p]
        out_c = o4.transpose(1, 3, 4, 2, 0).reshape(npair * 2 * 128, H)
        outs.append(out_c)
    out = np.concatenate(outs, axis=0)
    out += bc_h[None, :]
    return out, res


def kernel(**inputs):
    out, _ = run(inputs, trace=False)
    return out


# revision 34
# speedup vs baseline: 1.0189x; 1.0139x over previous
"""Trainium2 Bass kernel for nn_AttentionFusion (B=8192, M=4, H=1024), 8-core data parallel.

Math (exact reformulation of the reference):
  logits[b,m,n] = conf[b,m] * (y_{4b+m} . x_{4b+n}) + conf[b,m]*beta[4b+n]
      (alpha/d rank-1 terms dropped: softmax over n is shift-invariant)
      with Y = X G, G = (Wq/sqrt(H))^T Wk, beta = X (Wk^T bq/32)
  wt[b,n] = sum_m softmax_n(logits)[b,m,n]
  Z[b]    = sum_n wt[b,n] X[4b+n]
  out[b]  = Z[b] (Wo Wv / 4)^T  (+ bias, added on host)

v3: score path (Y = X G, gram S = Y X^T) in fp8 e4m3 with DoubleRow double-pumping;
value path in fp16 (STT convex combine on the DVE with weights straight out of softmax,
XBAR-DMA transpose of Z, fp16 output projection paired across super-tiles).
All DRAM tensors are host-prearranged so every streaming DMA is one contiguous
burst per partition, and DMA kicks are spread across the SP / Activation / Pool
queues (each kick costs ~0.6 us of queue time).
"""
import sys

if '/opt/trn_rl_repo' not in sys.path:
    sys.path.insert(0, '/opt/trn_rl_repo')

import numpy as np
import ml_dtypes

B, M, H = 8192, 4, 1024
NCORES = 8
B_CORE = B // NCORES            # 1024 batch rows per core
T_CORE = B_CORE * M             # 4096 tokens per core
T_SUPER = 512                   # tokens per super-tile (128 batch rows)
P = 128
OC = H // P                     # 8 output chunks
HC = H // P                     # 8 contraction chunks
F8 = ml_dtypes.float8_e4m3      # TRN e4m3: max normal 240
F16 = np.float16

_NC_CACHE = {}


def build_bass(n_super=T_CORE // T_SUPER):
    import concourse.bass as bass
    import concourse.mybir as mybir
    import concourse.tile as tile
    from concourse import bacc

    assert n_super % 2 == 0
    n_pair = n_super // 2
    t_core = n_super * T_SUPER
    b_core = t_core // M
    b_super = T_SUPER // M                 # 128 batch rows per super-tile

    nc = bacc.Bacc(None, target_bir_lowering=False)
    # all host-prearranged to partition-major contiguous layouts
    xT4 = nc.dram_tensor("xT4", [P, n_super, HC, T_SUPER], mybir.dt.float8e4,
                         kind="ExternalInput")
    xg = nc.dram_tensor("xg", [b_core, M * H], mybir.dt.float16, kind="ExternalInput")
    wg2 = nc.dram_tensor("wg2", [P, OC, HC, P], mybir.dt.float8e4,
                         kind="ExternalInput")
    wc2 = nc.dram_tensor("wc2", [P, OC, HC, P], mybir.dt.float16,
                         kind="ExternalInput")
    conf2 = nc.dram_tensor("conf2", [P, n_super, M], mybir.dt.float32,
                           kind="ExternalInput")
    cb2 = nc.dram_tensor("cb2", [P, n_super, 16], mybir.dt.float32,
                         kind="ExternalInput")
    syv = nc.dram_tensor("syv", [1], mybir.dt.float32, kind="ExternalInput")
    outT3 = nc.dram_tensor("outT3", [P, n_pair, OC, 2 * b_super], mybir.dt.float16,
                           kind="ExternalOutput")

    FDT = mybir.dt.float32
    DT16 = mybir.dt.float16
    DT8 = mybir.dt.float8e4
    BDT = mybir.dt.bfloat16
    AX = mybir.AxisListType.X
    MUL = mybir.AluOpType.mult
    ADD = mybir.AluOpType.add
    DR = mybir.MatmulPerfMode.DoubleRow
    COPY = mybir.ActivationFunctionType.Copy
    EXP = mybir.ActivationFunctionType.Exp

    from contextlib import ExitStack
    with tile.TileContext(nc) as tc:
        with ExitStack() as _es:
            wp = _es.enter_context(tc.tile_pool(name="wp", bufs=1))
            cp = _es.enter_context(tc.tile_pool(name="cp", bufs=1))
            xp = _es.enter_context(tc.tile_pool(name="xp", bufs=5))
            xgp = _es.enter_context(tc.tile_pool(name="xgp", bufs=5))
            yp = _es.enter_context(tc.tile_pool(name="yp", bufs=4))
            gpl = _es.enter_context(tc.tile_pool(name="gp", bufs=4))
            smp = _es.enter_context(tc.tile_pool(name="smp", bufs=4))
            zp = _es.enter_context(tc.tile_pool(name="zp", bufs=4))
            ztp = _es.enter_context(tc.tile_pool(name="ztp", bufs=3))
            osb = _es.enter_context(tc.tile_pool(name="osb", bufs=4))
            psp = _es.enter_context(tc.tile_pool(name="psp", bufs=3, space="PSUM"))
            psg = _es.enter_context(tc.tile_pool(name="psg", bufs=2, space="PSUM"))
            pso = _es.enter_context(tc.tile_pool(name="pso", bufs=3, space="PSUM"))
            drg = _es.enter_context(tc.tile_pool(name="drg", bufs=n_super, space="DRAM"))

            # ---- HAM warmup: dataless matmuls spin the PE to the warm clock
            #      while the lead-in DMAs stream ----
            wu = wp.tile([P, P], BDT, tag="warm", name="warm_sb")
            nc.vector.memset(wu[:], 1.0)
            wups = psg.tile([P, P], FDT, tag="gram_ps", name="warm_ps")
            for i in range(28):
                nc.tensor.matmul(wups[:], wu[:], wu[:],
                                 start=(i == 0), stop=(i == 27))
            wuo = wp.tile([P, P], FDT, tag="warmo", name="warm_out")
            nc.scalar.copy(wuo[:], wups[:])

            # ---- resident weights / constants ----
            # wg chunk 0 first: the first Y matmul only needs it + xt(0); the
            # remaining wg chunks are kicked after the first x loads
            wg_sb = wp.tile([P, OC, HC, P], DT8, tag="wg", name="wg_sb")
            wc_sb = wp.tile([P, OC, HC, P], DT16, tag="wc", name="wc_sb")
            nc.sync.dma_start(wg_sb[:, 0], wg2[:, 0])
            conf_sb = cp.tile([P, n_super, M], FDT, tag="conf", name="conf_sb")
            nc.scalar.dma_start(conf_sb[:], conf2[:])
            cb_sb = cp.tile([P, n_super, 16], FDT, tag="cb", name="cb_sb")
            nc.scalar.dma_start(cb_sb[:], cb2[:])
            sy_sb = cp.tile([P, 1], FDT, tag="sy", name="sy_sb")
            nc.scalar.dma_start(sy_sb[:], syv[:].partition_broadcast(P))

            def load_xt(s):
                # single kicks; the framework splits big DMAs across all queues
                xt = xp.tile([P, HC, T_SUPER], DT8, tag="xt")
                nc.sync.dma_start(xt[:], xT4[:, s])
                return xt

            def load_xg(s):
                xgt = xgp.tile([P, M * H], DT16, tag="xg")
                nc.scalar.dma_start(xgt[:], xg[s * b_super:(s + 1) * b_super])
                return xgt

            def y_proj(s, xt):
                """Y = X G in fp8 DoubleRow; quantize back to fp8 with scale sy."""
                yT = yp.tile([P, OC, T_SUPER], DT8, tag="yT")
                for oc in range(OC):
                    pt = psp.tile([P, T_SUPER], FDT, tag="proj")
                    for kk in range(HC // 2):
                        nc.tensor.matmul(
                            pt[:], wg_sb[:, oc, 2 * kk:2 * kk + 2, :],
                            xt[:, 2 * kk:2 * kk + 2, :],
                            start=(kk == 0), stop=(kk == HC // 2 - 1),
                            perf_mode=DR)
                    nc.scalar.activation(yT[:, oc, :], pt[:], COPY,
                                         scale=sy_sb[:])
                return yT

            def gram_softmax(s, xt, yT, xgt, zb2):
                """Block-diag scores -> softmax -> STT combine into zb2[:, s%2]."""
                gram_sb = gpl.tile([P, 4, P], FDT, tag="gram")
                for tt in range(4):
                    gps = psg.tile([P, P], FDT, tag="gram_ps")
                    tsl = slice(tt * P, (tt + 1) * P)
                    for kk in range(HC // 2):
                        nc.tensor.matmul(
                            gps[:], yT[:, 2 * kk:2 * kk + 2, tsl],
                            xt[:, 2 * kk:2 * kk + 2, tsl],
                            start=(kk == 0), stop=(kk == HC // 2 - 1),
                            perf_mode=DR)
                    nc.vector.tensor_copy(gram_sb[:, tt, :], gps[:])
                # bounce to DRAM with a 129-row tile pitch: 129*128 = 32*516, so
                # the (tile j, group g) diagonal walk becomes ONE uniform stride
                # of 516 and the whole 4x4-block gather is a single 3-dim DMA
                gram_dr = drg.tile([4, P + 1, P], FDT, tag="gram_dr",
                                   name=f"gram_dr{s}")
                nc.gpsimd.dma_start(gram_dr[:, 0:P, :].transpose([1, 0, 2]),
                                    gram_sb[:])
                s_sb = smp.tile([P, 16], FDT, tag="s", name="s_sb")
                base = gram_dr[:]
                src = bass.AP(base.tensor, base.offset,
                              [[4 * P + 4, P], [P, 4], [1, 4]])
                nc.gpsimd.dma_start(
                    s_sb[:].rearrange("p (m n) -> p m n", n=4), src)
                # logits = s_raw*conf_dev + conf*beta
                scl = smp.tile([P, 16], FDT, tag="scl", name="scl")
                for m in range(M):
                    nc.vector.scalar_tensor_tensor(
                        scl[:, 4 * m:4 * m + 4], s_sb[:, 4 * m:4 * m + 4],
                        conf_sb[:, s, m:m + 1], cb_sb[:, s, 4 * m:4 * m + 4],
                        op0=MUL, op1=ADD)
                ex = smp.tile([P, 16], FDT, tag="ex", name="ex")
                nc.scalar.activation(ex[:], scl[:], EXP)
                z4 = smp.tile([P, M], FDT, tag="z4", name="z4")
                nc.vector.reduce_sum(z4[:], ex[:].rearrange("p (m n) -> p m n", n=4),
                                     axis=AX)
                r4 = smp.tile([P, M], FDT, tag="r4", name="r4")
                nc.vector.reciprocal(r4[:], z4[:])
                w4 = smp.tile([P, M], FDT, tag="w4", name="w4")
                nc.vector.tensor_scalar_mul(w4[:], ex[:, 0:4], r4[:, 0:1])
                for m in range(1, M):
                    nc.vector.scalar_tensor_tensor(
                        w4[:], ex[:, 4 * m:4 * m + 4], r4[:, m:m + 1], w4[:],
                        op0=MUL, op1=ADD)
                # convex combine, batch-major, fp16 accumulate (eps ~1e-3)
                with nc.allow_low_precision(reason="convex combine, fp16 acc"):
                    zb = zb2[:, s % 2, :]
                    nc.vector.tensor_scalar_mul(zb, xgt[:, 0:H], w4[:, 0:1])
                    for n in range(1, M):
                        nc.vector.scalar_tensor_tensor(
                            zb, xgt[:, n * H:(n + 1) * H], w4[:, n:n + 1],
                            zb, op0=MUL, op1=ADD)

            def transpose_pair(zb2):
                zT2 = ztp.tile([P, 16, P], DT16, tag="zT2")
                nc.sync.dma_start_transpose(zT2[:], zb2[:].rearrange("p a b -> p (a b)"))
                # logical row r = pair*1024 + h  ->  partition h%128, chunk pair*8 + h//128
                return zT2

            def transpose_half(zb2, half, zT2=None):
                if zT2 is None:
                    zT2 = ztp.tile([P, 16, P], DT16, tag="zT2")
                nc.sync.dma_start_transpose(zT2[:, 8 * half:8 * (half + 1), :],
                                            zb2[:, half, :])
                return zT2

            def out_pair(pr, zT2):
                zv = zT2[:].rearrange("p (a c) b -> p c a b", a=2)
                o_sb = osb.tile([P, OC, 2 * b_super], DT16, tag="osb")
                for oc in range(OC):
                    po = pso.tile([P, 2 * b_super], FDT, tag="outp")
                    for hc in range(HC):
                        nc.tensor.matmul(
                            po[:], wc_sb[:, oc, hc, :], zv[:, hc],
                            start=(hc == 0), stop=(hc == HC - 1))
                    if oc % 2 == 0:
                        nc.scalar.copy(o_sb[:, oc, :], po[:])
                    else:
                        nc.vector.tensor_copy(o_sb[:, oc, :], po[:])
                    if oc == OC // 2 - 1:
                        # stream the first half out while the rest computes
                        nc.sync.dma_start(outT3[:, pr, 0:OC // 2], o_sb[:, 0:OC // 2])
                nc.sync.dma_start(outT3[:, pr, OC // 2:], o_sb[:, OC // 2:])

            # software pipeline: loads prefetched one super-tile ahead; the
            # transpose / output-write kicks are deferred so their data is a
            # full super-tile old by the time the SP queue reaches them (no
            # head-of-line blocking of the next x loads)
            xts = {0: load_xt(0)}
            xgs = {0: load_xg(0)}
            for oc in range(1, OC):
                nc.sync.dma_start(wg_sb[:, oc], wg2[:, oc])
            # wc needed first by out_pair(0) at iteration 4: spread the 2MB
            # load across iterations 1..4 to not crowd the x streams
            wc_iters = min(4, n_super - 1)
            wc_per = -(-OC // wc_iters)
            state = {}
            zbs = {}
            zts = {}
            for s in range(n_super):
                if s + 1 < n_super:
                    xts[s + 1] = load_xt(s + 1)
                yT = y_proj(s, xts[s])
                state[s] = (xts.pop(s), yT, xgs.pop(s))
                if 1 <= s <= wc_iters:
                    for oc in range(wc_per * (s - 1), min(OC, wc_per * s)):
                        nc.scalar.dma_start(wc_sb[:, oc], wc2[:, oc])
                if s >= 1:
                    sp = s - 1
                    if sp % 2 == 0:
                        zbs[sp // 2] = zp.tile([P, 2, H], DT16, tag="zb2",
                                               name=f"zb2_{sp // 2}")
                    gram_softmax(sp, *state.pop(sp), zbs[sp // 2])
                if s >= 3 and s % 2 == 1:
                    pr = (s - 3) // 2
                    zts[pr] = transpose_pair(zbs.pop(pr))
                if s >= 4 and s % 2 == 0:
                    pr = (s - 4) // 2
                    out_pair(pr, zts.pop(pr))
                if s + 1 < n_super:
                    xgs[s + 1] = load_xg(s + 1)
            sl = n_super - 1
            # last pair: transpose the even half as soon as its combine is done
            # so only the odd half's transpose sits on the tail critical path
            ztl = transpose_half(zbs[sl // 2], 0)
            gram_softmax(sl, *state.pop(sl), zbs[sl // 2])
            transpose_half(zbs.pop(sl // 2), 1, ztl)
            if n_pair >= 2:
                out_pair(n_pair - 2, zts.pop(n_pair - 2))
            out_pair(n_pair - 1, ztl)
    nc.compile()
    return nc


def _get_nc(n_super=T_CORE // T_SUPER):
    if n_super not in _NC_CACHE:
        _NC_CACHE[n_super] = build_bass(n_super)
    return _NC_CACHE[n_super]


def prep_in_maps(inputs, ncores=NCORES):
    """Host-side: fold weights, pick fp8 scales, prearrange layouts, cast."""
    f32 = np.float32
    f64 = np.float64
    feats = np.asarray(inputs["features"], f32)
    confs = np.asarray(inputs["confidences"], f32).reshape(-1, M)
    Wq = np.asarray(inputs["Wq"], f64)
    Wk = np.asarray(inputs["Wk"], f64)
    Wv = np.asarray(inputs["Wv"], f64)
    Wo = np.asarray(inputs["Wo"], f64)
    bq = np.asarray(inputs["bq"], f64)
    bv = np.asarray(inputs["bv"], f64)
    bo = np.asarray(inputs["bo"], f64)

    s = 1.0 / np.sqrt(H)
    G = (Wq * s).T @ Wk                         # [h, h']
    WcT = ((Wo @ Wv) / 4.0).T                   # [f, o]
    bc_h = (bv @ Wo.T + bo).astype(f32)         # added on host after the run
    w_vec = Wk.T @ (bq * s)                     # beta = X w_vec

    nb = feats.shape[0]
    b_core = nb // ncores
    t_core = b_core * M
    n_super = t_core // T_SUPER
    X = feats.reshape(nb * M, H)

    sx = f32(224.0 / np.abs(X).max())
    sg = f32(224.0 / np.abs(G).max())
    X8 = (X * sx).astype(F8)                    # [t, h]
    wg_f8 = (G * sg).astype(F8)                 # [k, o]
    col_sig = np.sqrt((np.asarray(wg_f8, f32) ** 2).sum(axis=0)).max() * sx
    sy = f32(224.0 / (6.5 * col_sig))           # fp8-Y sigma ~34, 6.5-sigma headroom
    descale = f32(1.0) / (f32(sx) * f32(sx) * f32(sg) * f32(sy))

    # prearranged weights: [p, oc, c, o'] with k = c*128+p, o = oc*128+o'
    wg2_h = np.ascontiguousarray(
        wg_f8.reshape(HC, P, OC, P).transpose(1, 2, 0, 3))
    wc2_h = np.ascontiguousarray(
        WcT.astype(F16).reshape(HC, P, OC, P).transpose(1, 2, 0, 3))

    beta = (X @ w_vec.astype(f32)).astype(f32)  # [nb*M]
    conf_dev = confs * descale
    cbeta = confs[:, :, None] * beta.reshape(nb, M)[:, None, :]   # [b, m, n]
    cb16_h = cbeta.reshape(nb, 16).astype(f32)
    xg_h = feats.reshape(nb, M * H).astype(F16)

    in_maps = []
    for c in range(ncores):
        tsl = slice(c * t_core, (c + 1) * t_core)
        bsl = slice(c * b_core, (c + 1) * b_core)
        # xT4 [p, s, c, t]: X8[(s t), (c p)] -> transpose
        xT4_h = np.ascontiguousarray(
            X8[tsl].reshape(n_super, T_SUPER, HC, P).transpose(3, 0, 2, 1))
        conf2_h = np.ascontiguousarray(
            conf_dev[bsl].reshape(n_super, P, M).transpose(1, 0, 2))
        cb2_h = np.ascontiguousarray(
            cb16_h[bsl].reshape(n_super, P, 16).transpose(1, 0, 2))
        in_maps.append({
            "xT4": xT4_h,
            "xg": np.ascontiguousarray(xg_h[bsl]),
            "wg2": wg2_h, "wc2": wc2_h,
            "conf2": conf2_h, "cb2": cb2_h,
            "syv": np.array([sy], f32),
        })
    return in_maps, bc_h


def install_ntff_hook():
    """Best-effort shim so run_bass_kernel_spmd(trace=True) can profile under axon."""
    import types
    try:
        from antenv.axon_hooks import get_axon_ntff_profile_hook  # noqa: F401
        return True
    except ImportError:
        pass
    try:
        import antenv
        mod = types.ModuleType("antenv.axon_hooks")
        _state = {"hook": None}
        mod.set_axon_ntff_profile_hook = lambda h: _state.__setitem__("hook", h)
        mod.get_axon_ntff_profile_hook = lambda: _state["hook"]
        sys.modules["antenv.axon_hooks"] = mod
        antenv.axon_hooks = mod
        from trn_agent_boot.trn_boot import _ntff_profile_via_ctypes
        hook = _ntff_profile_via_ctypes('/opt/axon/libaxon_pjrt.so')
        if hook is None:
            return False
        mod.set_axon_ntff_profile_hook(hook)
        return True
    except Exception:
        return False


def run(inputs, trace=False, tmpdir=None):
    """Run the 8-core kernel; returns (out [B, H] f32, BassKernelResults)."""
    from concourse.bass_utils import run_bass_kernel_spmd
    nc = _get_nc()
    in_maps, bc_h = prep_in_maps(inputs)
    if trace:
        install_ntff_hook()
    res = run_bass_kernel_spmd(nc, in_maps, core_ids=list(range(NCORES)),
                               trace=trace, tmpdir=tmpdir)
    # outT3 [p, pr, oc, b2]: out[b, o] with b = (2pr + b2//128)*128 + b2%128,
    # o = oc*128 + p
    outs = []
    for o in res.results:
        o3 = np.asarray(o["outT3"], np.float32)     # [128, n_pair, 8, 256]
        npair = o3.shape[1]
        o4 = o3.reshape(P, npair, OC, 2, 128)       # [p, pr, oc, half, 128]
        # -> [pr, half, b(128), oc, p]
        out_c = o4.transpose(1, 3, 4, 2, 0).reshape(npair * 2 * 128, H)
        outs.append(out_c)
    out = np.concatenate(outs, axis=0)
    out += bc_h[None, :]
    return out, res


def kernel(**inputs):
    out, _ = run(inputs, trace=False)
    return out
